# revision 1
# baseline (speedup 1.0000x reference)
"""Trainium2 Bass kernel for CRF log-likelihood (B=128, S=512, U=1024, T=48).

Strategy (data-parallel, 16 batch rows per core, no collectives):
  - Emissions scores = H @ W computed on PE (K=1024 in 8 chunks of 128),
    H streamed from HBM with U on partitions (fully contiguous reads).
  - Forward algorithm in exp space: one (49x49)@(49x16) PE matmul + one
    DVE multiply per time step.  A 49th "done" state absorbs finished rows
    (transition column = exp(end_transitions)), driven purely by per-core
    data masks, so all cores run the identical SPMD program.
  - A constant per-step normalizer exp(-C0) keeps fp32 in range; corrected
    on the host via + C0*(s_len-1).
  - The chain is split into a forward scan (steps 1..255) and an
    independent backward scan (steps 511..256) that run concurrently,
    halving the sequential latency.  Z = sum_j alpha_cut[j]*beta_cut[j].
  - Gold-path emission sum (numerator) on device via a host-built
    onehot*mask multiply + reduce against the same score tiles.
  - Tiny O(B*S) gathers of the small parameter tensors (transition/start/
    end terms of the numerator, final log/assembly) happen on the host.
"""

import os

import numpy as np

import concourse.bass as bass
import concourse.tile as tile
from concourse import bacc, mybir
from concourse.bass_utils import run_bass_kernel_spmd

B, S, U, T = 128, 512, 1024, 48
NCORES = 8
NB = B // NCORES          # 16 rows per core
NPOS = NB * S             # 8192 positions per core, pos = s*NB + b
TA = T + 1                # 49 states (48 tags + "done")
CUT = 261                 # fwd computes alpha_1..alpha_CUT, bwd beta_510..beta_CUT
C0 = 4.8                  # per-step log-space normalizer
SCHUNK = 32               # time steps per emission chunk
NCHUNK = S // SCHUNK      # 8
CPOS = SCHUNK * NB        # 1024 positions per chunk -> 2 PSUM halves of 512
NEG = -1.0e9              # pad logit; exp(NEG) == 0 in fp32
F32 = mybir.dt.float32
BF16 = mybir.dt.bfloat16
F16 = mybir.dt.float16
NEGH = -60000.0           # fp16-representable pad logit; exp() == 0

_PROGRAM = None  # compiled program cache
LAST_EXEC_NS = None
LAST_RESULT = None



def _build_program():
    nc = bacc.Bacc("TRN2", target_bir_lowering=False, debug=False,
                   enable_asserts=False)

    def din(name, shape, dt=F32):
        return nc.dram_tensor(name, list(shape), dt, kind="ExternalInput").ap()

    def dout(name, shape):
        return nc.dram_tensor(name, list(shape), F32, kind="ExternalOutput").ap()

    h = din("h", (U, S, NB), F16)  # host-pretransposed
    w = din("w", (U, TA), F16)  # 49th col zero
    lhs_fwd = din("lhs_fwd", (TA, TA), BF16)  # Ahat
    lhs_bwd = din("lhs_bwd", (TA, TA), BF16)  # Ahat^T
    ones_k1 = din("ones_k1", (1, TA), F16)  # [1]*48 + [-1]
    ones49 = din("ones49", (TA, 1), BF16)
    padflag = din("padflag", (1, NPOS), F16)  # {0, NEGH}
    msel = din("msel", (TA, NPOS), F16)     # onehot(tag)*wmask, row48=0
    bias_e = din("bias_e", (TA, 1))         # [b - C0; NEGb]
    bias_a0 = din("bias_a0", (TA, 1))       # [b + start; NEG]
    beta_init = din("beta_init", (TA, NB), BF16)  # [exp(end); 1]

    z_out = dout("z_out", (1, NB))
    prod_out = dout("prod", (TA, NPOS))

    with tile.TileContext(nc) as tc:
        with (
            tc.tile_pool(name="consts", bufs=1) as consts,
            tc.tile_pool(name="hpool", bufs=8) as hpool,
            tc.tile_pool(name="tmp", bufs=2) as tmpp,
            tc.tile_pool(name="epsum", bufs=2, space="PSUM") as epsum,
            tc.tile_pool(name="psA", bufs=2, space="PSUM") as psA,
            tc.tile_pool(name="psB", bufs=2, space="PSUM") as psB,
            tc.tile_pool(name="psZ", bufs=1, space="PSUM") as psZ,
            tc.tile_pool(name="sA", bufs=2) as sAp,
            tc.tile_pool(name="sB", bufs=2) as sBp,
        ):
            # ---- constants into SBUF ----
            w_sb = consts.tile([128, 8 * TA], F16, tag="w_sb")
            nc.sync.dma_start(w_sb[:].rearrange("p (c t) -> p c t", c=8),
                              w.rearrange("(c p) t -> p c t", p=128))
            lhsf_sb = consts.tile([TA, TA], BF16, tag="lhsf")
            nc.gpsimd.dma_start(lhsf_sb[:], lhs_fwd)
            lhsb_sb = consts.tile([TA, TA], BF16, tag="lhsb")
            nc.gpsimd.dma_start(lhsb_sb[:], lhs_bwd)
            ones1_sb = consts.tile([1, TA], F16, tag="ones1")
            nc.gpsimd.dma_start(ones1_sb[:], ones_k1)
            ones49_sb = consts.tile([TA, 1], BF16, tag="ones49v")
            nc.gpsimd.dma_start(ones49_sb[:], ones49)
            pad_sb = consts.tile([1, NPOS], F16, tag="pad")
            nc.scalar.dma_start(pad_sb[:], padflag)
            msel_sb = consts.tile([TA, NPOS], F16, tag="msel")
            bias_e_sb = consts.tile([TA, 1], F32, tag="bias_e")
            nc.gpsimd.dma_start(bias_e_sb[:], bias_e)
            bias_a0_sb = consts.tile([TA, 1], F32, tag="bias_a0")
            nc.gpsimd.dma_start(bias_a0_sb[:], bias_a0)
            beta0_sb = consts.tile([TA, NB], BF16, tag="beta0")
            nc.gpsimd.dma_start(beta0_sb[:], beta_init)

            escan = consts.tile([TA, NPOS], F32, tag="escan")
            alpha0_sb = consts.tile([TA, NB], BF16, tag="alpha0")

            hs_tiles = {}

            def dma_chunk(c):
                hs = hpool.tile([128, CPOS * 8], F16, tag="hs", name="hs")
                hs_tiles[c] = hs
                for hh in range(8):
                    src = h[hh * 128:(hh + 1) * 128,
                            c * SCHUNK:(c + 1) * SCHUNK, :].rearrange(
                        "p s b -> p (s b)")
                    (nc.sync if hh % 2 == 0 else nc.gpsimd).dma_start(
                        hs[:, hh * CPOS:(hh + 1) * CPOS], src)
                nc.scalar.dma_start(msel_sb[:, c * CPOS:(c + 1) * CPOS],
                                    msel[:, c * CPOS:(c + 1) * CPOS])

            def chunk_compute_ops(c):
                """Small closures, emitted one per chain step."""
                hs = lambda: hs_tiles[c]
                state = {}
                ops = []

                def mk_mm(hh):
                    def f():
                        if hh == 0:
                            state[0] = epsum.tile([TA, 512], F32, tag="eps", name="eps")
                        ps = state[0]
                        off = hh * CPOS
                        nc.tensor.matmul(ps[:], w_sb[:, hh * TA:(hh + 1) * TA],
                                         hs()[:, off:off + 512],
                                         start=(hh == 0), stop=False)
                    return f

                def mk_pad():
                    def f():
                        ps = state[0]
                        pos0 = c * CPOS
                        nc.tensor.matmul(ps[:], ones1_sb[:],
                                         pad_sb[:, pos0:pos0 + 512],
                                         start=False, stop=True)
                    return f

                def mk_tail():
                    def f():
                        ps = state[0]
                        pos0 = c * CPOS
                        nc.scalar.activation(escan[:, pos0:pos0 + 512], ps[:],
                                             mybir.ActivationFunctionType.Exp,
                                             bias=bias_e_sb[:])
                        if c == 0:
                            nc.scalar.activation(alpha0_sb[:], ps[:, 0:NB],
                                                 mybir.ActivationFunctionType.Exp,
                                                 bias=bias_a0_sb[:])
                        state[1] = tmpp.tile([TA, 512], F32, tag="ptmp", name="ptmp")
                    return f

                def mk_num(q):
                    def f():
                        ps = state[0]
                        pt = state[1]
                        pos0 = c * CPOS
                        nc.vector.tensor_tensor(
                            pt[:, q * 128:(q + 1) * 128],
                            ps[0:TA, q * 128:(q + 1) * 128],
                            msel_sb[:, pos0 + q * 128:pos0 + (q + 1) * 128],
                            mybir.AluOpType.mult)
                    return f

                def mk_prod_dma():
                    def f():
                        nc.scalar.dma_start(prod_out[:, c * CPOS:(c + 1) * CPOS],
                                            state[1][:])
                    return f

                for hh in range(8):
                    ops.append(mk_mm(hh))
                ops.append(mk_pad())
                ops.append(mk_tail())
                for q in range(4):
                    ops.append(mk_num(q))
                ops.append(mk_prod_dma())
                return ops

            # ---- schedules ----
            npair = NCHUNK // 2
            for p in range(3):
                dma_chunk(p)
                dma_chunk(NCHUNK - 1 - p)
            for op_pair in zip(chunk_compute_ops(0), chunk_compute_ops(NCHUNK - 1)):
                for op in op_pair:
                    op()

            dma_sched = {}
            comp_sched = {}
            for p in range(3, npair):
                dma_sched.setdefault(SCHUNK * (p - 1) - 16, []).extend(
                    (p, NCHUNK - 1 - p))
            for p in range(1, npair):
                ops_a = chunk_compute_ops(p)
                ops_b = chunk_compute_ops(NCHUNK - 1 - p)
                inter = [op for pair in zip(ops_a, ops_b) for op in pair]
                start = max(2, SCHUNK * p - 34)
                for j, op in enumerate(inter):
                    comp_sched.setdefault(start + j, []).append(op)

            # ---- the two scan chains, interleaved ----
            alpha = alpha0_sb
            beta = beta0_sb
            for i in range(CUT):
                for c in dma_sched.get(i, ()):
                    dma_chunk(c)
                for op in comp_sched.get(i, ()):
                    op()
                s_f = 1 + i
                pa = psA.tile([TA, NB], F32, tag="pa")
                nc.tensor.matmul(pa[:], lhsf_sb[:], alpha[:], start=True, stop=True)
                na = sAp.tile([TA, NB], BF16, tag="na")
                nc.vector.tensor_tensor(na[:], pa[:],
                                        escan[:, s_f * NB:(s_f + 1) * NB],
                                        mybir.AluOpType.mult)
                alpha = na

                if i < S - 2 - CUT:
                    s_b = S - 1 - i
                    rb = sBp.tile([TA, NB], BF16, tag="rb")
                    nc.vector.tensor_tensor(rb[:], beta[:],
                                            escan[:, s_b * NB:(s_b + 1) * NB],
                                            mybir.AluOpType.mult)
                    pb = psB.tile([TA, NB], F32, tag="pb")
                    nc.tensor.matmul(pb[:], lhsb_sb[:], rb[:], start=True, stop=True)
                    beta = pb

            # final bwd step: s_b = CUT+1 = 256 -> beta_255
            rb = sBp.tile([TA, NB], BF16, tag="rb")
            nc.vector.tensor_tensor(rb[:], beta[:],
                                    escan[:, (CUT + 1) * NB:(CUT + 2) * NB],
                                    mybir.AluOpType.mult)
            pb = psB.tile([TA, NB], F32, tag="pb")
            nc.tensor.matmul(pb[:], lhsb_sb[:], rb[:], start=True, stop=True)

            # ---- readout: z = sum_j alpha_cut[j] * beta_cut[j] ----
            g = sAp.tile([TA, NB], BF16, tag="gamma")
            nc.vector.tensor_tensor(g[:], pb[:], alpha[:], mybir.AluOpType.mult)
            zp = psZ.tile([1, NB], F32, tag="zp")
            nc.tensor.matmul(zp[:], ones49_sb[:], g[:], start=True, stop=True)
            zsb = consts.tile([1, NB], F32, tag="zsb")
            nc.vector.tensor_copy(zsb[:], zp[:])
            nc.sync.dma_start(z_out, zsb[:])

    nc.compile()
    return nc


def _host_inputs(H, W, bb, st, en, tr, tag, s_len, w_mask):
    """Build the per-core input maps (all f32)."""
    import ml_dtypes
    BF = ml_dtypes.bfloat16
    A = np.exp(tr.astype(np.float64)).astype(np.float32)
    Ahat = np.zeros((TA, TA), np.float32)
    Ahat[:T, :T] = A
    Ahat[:T, T] = np.exp(en).astype(np.float32)
    Ahat[T, T] = 1.0

    beta_init = np.zeros((TA, NB), np.float32)
    beta_init[:T, :] = np.exp(en).astype(np.float32)[:, None]
    beta_init[T, :] = 1.0
    NEGb = np.float32(np.float16(NEGH))  # fp16 pad logit (exact cancel)

    Wp = np.zeros((U, TA), np.float16)
    Wp[:, :T] = W.astype(np.float16)
    ones_k1 = np.ones((1, TA), np.float16)
    ones_k1[0, T] = -1.0
    shared = {
        "w": Wp,
        "lhs_fwd": Ahat.astype(BF),
        "lhs_bwd": np.ascontiguousarray(Ahat.T).astype(BF),
        "ones_k1": ones_k1,
        "ones49": np.ones((TA, 1), BF),
        "bias_e": np.concatenate([(bb - C0).astype(np.float32),
                                  [NEGb]]).reshape(TA, 1),
        "bias_a0": np.concatenate([(bb + st).astype(np.float32),
                                   [np.float32(NEG)]]).reshape(TA, 1),
        "beta_init": beta_init.astype(BF),
    }

    s_idx = np.arange(S)
    in_maps = []
    for k in range(NCORES):
        rows = slice(k * NB, (k + 1) * NB)
        tag_l = tag[rows]            # (NB, S)
        len_l = s_len[rows]          # (NB,)
        wm_l = w_mask[rows]          # (NB, S)
        pad = (s_idx[None, :] >= len_l[:, None])          # (NB, S)
        padflag = np.where(pad, NEGb, np.float32(0.0)).T.reshape(1, NPOS).astype(np.float16)
        msel3 = np.zeros((TA, S, NB), np.float16)
        msel3[tag_l.T, s_idx[:, None], np.arange(NB)[None, :]] = wm_l.T
        im = dict(shared)
        im["h"] = np.ascontiguousarray(H[rows].transpose(2, 1, 0).astype(np.float16))
        im["padflag"] = np.ascontiguousarray(padflag)
        im["msel"] = np.ascontiguousarray(msel3.reshape(TA, NPOS))
        in_maps.append(im)
    return in_maps


def kernel(H, W, b, start_transitions, end_transitions, transitions,
           tag, s_len, w_mask):
    global _PROGRAM
    H = np.asarray(H, np.float32)
    W = np.asarray(W, np.float32)
    bb = np.asarray(b, np.float32)
    st = np.asarray(start_transitions, np.float32)
    en = np.asarray(end_transitions, np.float32)
    tr = np.asarray(transitions, np.float32)
    tag = np.asarray(tag)
    s_len = np.asarray(s_len)
    w_mask = np.asarray(w_mask, np.float32)

    if _PROGRAM is None:
        _PROGRAM = _build_program()
    nc = _PROGRAM

    in_maps = _host_inputs(H, W, bb, st, en, tr, tag, s_len, w_mask)
    trace = bool(int(os.environ.get("KERNEL_TRACE", "0")))
    r = run_bass_kernel_spmd(nc, in_maps, list(range(NCORES)), trace=trace,
                             tmpdir=os.environ.get("KERNEL_TRACE_DIR") or None)
    global LAST_EXEC_NS, LAST_RESULT
    LAST_RESULT = r
    LAST_EXEC_NS = r.exec_time_ns
    res = r.results

    z = np.concatenate([np.asarray(r["z_out"]).reshape(NB) for r in res])
    prod = np.stack([np.asarray(r["prod"]) for r in res])  # (NC, TA, NPOS)

    # ---- host assembly ----
    logZ = np.log(z.astype(np.float64)) + C0 * (s_len.astype(np.float64) - 1)
    num_emit = (prod.reshape(NCORES, TA, S, NB).sum(axis=(1, 2), dtype=np.float64)
                .reshape(B))
    bidx = np.arange(B)
    num = (st[tag[:, 0]].astype(np.float64)
           + num_emit
           + (bb[tag].astype(np.float64) * w_mask).sum(axis=1)
           + (tr[tag[:, :-1], tag[:, 1:]].astype(np.float64) * w_mask[:, 1:]).sum(axis=1)
           + en[tag[bidx, s_len - 1]].astype(np.float64))
    return (num - logZ).astype(np.float32)



# revision 6
# speedup vs baseline: 1.1395x; 1.1395x over previous
"""Trainium2 Bass kernel for CRF log-likelihood (B=128, S=512, U=1024, T=48).

Strategy (data-parallel, 16 batch rows per core, no collectives):
  - Emission scores H@W on PE (K=1024 in 8 chunks of 128), f16 in / f32 psum,
    in 32 position-tiles of 16 positions (N=256 matmuls) so each emission
    matmul hides inside the chain matmul's pipeline-drain window.
  - Forward+backward scans fused: ONE block-diagonal (113x113) matmul per
    round (fwd states on partitions 0:49, bwd on 64:113 for 64-alignment)
    followed by ONE (113,16) DVE multiply with exp'd scores.  The bwd half
    of H is time-reversed on the host so both chains read the same escan
    column block each round.  256 rounds instead of 511.
  - A 49th "done" state absorbs finished rows (pad logits via a rank-1
    matmul), constant normalizer exp(-C0) per step, corrected on host.
  - Numerator: host recovers score[tag] = log(escan) - (b - C0) from the
    exp'd-scores tensor which is DMA'd out; no extra device work.
"""

import os
from collections import deque

import numpy as np

import concourse.bass as bass
import concourse.tile as tile
from concourse import bacc, mybir
from concourse.bass_utils import run_bass_kernel_spmd

B, S, U, T = 128, 512, 1024, 48
NCORES = 8
NB = B // NCORES          # 16 rows per core
NPOS = NB * S             # 8192 positions per core (pos = p*NB + b)
TA = T + 1                # 49 states (48 tags + "done")
BOT = 64                  # partition base of the bwd chain lane
H113 = BOT + TA           # 113
HALF = S // 2             # 256 positions per direction
NTP = 16                  # positions per emission tile
CPT = NTP * NB            # 256 columns per emission tile
NPAIR = HALF // NTP       # 16 tile pairs
C0 = 4.8                  # per-step log-space normalizer
NEG = -1.0e9
NEGH = -60000.0           # fp16-representable pad logit; exp() == 0
F32 = mybir.dt.float32
BF16 = mybir.dt.bfloat16
F16 = mybir.dt.float16

_PROGRAM = None
LAST_EXEC_NS = None
LAST_RESULT = None


def _build_program():
    nc = bacc.Bacc("TRN2", target_bir_lowering=False, debug=False,
                   enable_asserts=False)

    def din(name, shape, dt=F32):
        return nc.dram_tensor(name, list(shape), dt, kind="ExternalInput").ap()

    def dout(name, shape):
        return nc.dram_tensor(name, list(shape), F32, kind="ExternalOutput").ap()

    h2 = din("h2", (U, S, NB), F16)          # host-permuted (bwd half flipped)
    w = din("w", (U, TA), F16)               # 49th col zero
    mhat = din("mhat", (H113, H113), BF16)   # blockdiag lhsT
    mfin = din("mfin", (H113, H113), BF16)   # final-round lhsT
    onesk = din("onesk", (1, TA), F16)       # [1]*48 + [-1]
    onesz = din("onesz", (TA, 1), BF16)
    padflag = din("padflag", (1, NPOS), F16)  # {0, NEGH}, permuted
    bias_e = din("bias_e", (TA, 1))          # [b - C0; NEGb]
    bias_a0 = din("bias_a0", (TA, 1))        # [b + start; NEG]
    beta_init = din("beta_init", (TA, NB), BF16)

    z_out = dout("z_out", (1, NB))
    e_top = dout("e_top", (TA, HALF * NB))
    e_bot = dout("e_bot", (TA, HALF * NB))

    with tile.TileContext(nc) as tc:
        with (
            tc.tile_pool(name="consts", bufs=1) as consts,
            tc.tile_pool(name="hpool", bufs=6) as hpool,
            tc.tile_pool(name="epsum", bufs=4, space="PSUM") as epsum,
            tc.tile_pool(name="psY", bufs=2, space="PSUM") as psY,
            tc.tile_pool(name="psZ", bufs=1, space="PSUM") as psZ,
            tc.tile_pool(name="xpool", bufs=2) as xpool,
        ):
            # ---- constants into SBUF ----
            w_sb = consts.tile([128, 8 * TA], F16, tag="w_sb")
            nc.scalar.dma_start(w_sb[:].rearrange("p (c t) -> p c t", c=8),
                                w.rearrange("(c p) t -> p c t", p=128))
            mhat_sb = consts.tile([H113, H113], BF16, tag="mhat")
            nc.scalar.dma_start(mhat_sb[:], mhat)
            mfin_sb = consts.tile([H113, H113], BF16, tag="mfin")
            nc.scalar.dma_start(mfin_sb[:], mfin)
            onesk_sb = consts.tile([1, TA], F16, tag="onesk")
            nc.scalar.dma_start(onesk_sb[:], onesk)
            onesz_sb = consts.tile([H113, 1], BF16, tag="onesz")
            nc.scalar.dma_start(onesz_sb[BOT:H113, :], onesz)
            padf_sb = consts.tile([1, NPOS], F16, tag="padf")
            nc.sync.dma_start(padf_sb[:], padflag)
            bias_e0_sb = consts.tile([TA, 1], F32, tag="bias_e0")
            nc.scalar.dma_start(bias_e0_sb[:], bias_e)
            bias_eb_sb = consts.tile([H113, 1], F32, tag="bias_eb")
            nc.scalar.dma_start(bias_eb_sb[BOT:H113, :], bias_e)
            bias_a0_sb = consts.tile([TA, 1], F32, tag="bias_a0")
            nc.scalar.dma_start(bias_a0_sb[:], bias_a0)
            beta_sb = consts.tile([H113, NB], BF16, tag="beta0")
            nc.scalar.dma_start(beta_sb[BOT:H113, :], beta_init)

            escan2 = consts.tile([H113, HALF * NB], F32, tag="escan2")
            # rows 49:64 must be zero; engine partition bases must be
            # 32-aligned, so clear 32:64 and let the fwd ACTs overwrite 32:49
            nc.gpsimd.memset(escan2[32:BOT, :], 0.0)
            x1 = consts.tile([H113, NB], BF16, tag="x1")
            nc.gpsimd.memset(x1[:], 0.0)

            hs_tiles = {}

            def dma_tile(t, q):
                """t in 0..31: fwd tile j = 2j, bwd tile j = 2j+1."""
                j = t // 2
                p0 = (HALF if t % 2 else 0) + j * NTP
                hs = hpool.tile([128, 8 * CPT], F16, tag="hs", name="hs")
                hs_tiles[t] = hs
                src = h2[:, p0:p0 + NTP, :].rearrange(
                    "(c p) s b -> p c (s b)", p=128)
                q.dma_start(hs[:].rearrange("p (c x) -> p c x", c=8), src)

            def em_ops(t):
                """Emission ops for tile t as a list of closures."""
                j = t // 2
                is_b = t % 2
                pos0 = ((HALF if is_b else 0) + j * NTP) * NB
                cols = slice(j * NTP * NB, j * NTP * NB + CPT)
                lo, hi = (BOT, H113) if is_b else (0, TA)
                state = {}
                ops = []

                def mk_mm(hh):
                    def f():
                        if hh == 0:
                            state[0] = epsum.tile([H113, CPT], F32, tag="eps",
                                                  name="eps")
                        ps = state[0]
                        nc.tensor.matmul(ps[lo:hi, :],
                                         w_sb[:, hh * TA:(hh + 1) * TA],
                                         hs_tiles[t][:, hh * CPT:(hh + 1) * CPT],
                                         start=(hh == 0), stop=False)
                    return f

                def mk_pad():
                    def f():
                        nc.tensor.matmul(state[0][lo:hi, :], onesk_sb[:],
                                         padf_sb[:, pos0:pos0 + CPT],
                                         start=False, stop=True)
                    return f

                def mk_act():
                    def f():
                        ps = state[0]
                        if is_b:
                            nc.scalar.activation(
                                escan2[BOT:H113, cols], ps[BOT:H113, :],
                                mybir.ActivationFunctionType.Exp,
                                bias=bias_eb_sb[BOT:H113, :])
                            if t == 1:
                                nc.vector.tensor_tensor(
                                    x1[BOT:H113, :], beta_sb[BOT:H113, :],
                                    escan2[BOT:H113, 0:NB],
                                    mybir.AluOpType.mult)
                        else:
                            nc.scalar.activation(
                                escan2[0:TA, cols], ps[0:TA, :],
                                mybir.ActivationFunctionType.Exp,
                                bias=bias_e0_sb[:])
                            if t == 0:
                                nc.scalar.activation(
                                    x1[0:TA, :], ps[0:TA, 0:NB],
                                    mybir.ActivationFunctionType.Exp,
                                    bias=bias_a0_sb[:])
                    return f

                for hh in range(8):
                    ops.append(mk_mm(hh))
                ops.append(mk_pad())
                ops.append(mk_act())
                return ops

            # ---- pre-phase: tiles 0..3 (pairs 0 and 1) ----
            dma_tile(0, nc.sync)
            dma_tile(1, nc.gpsimd)
            dma_tile(2, nc.sync)
            dma_tile(3, nc.gpsimd)
            for op_pair in zip(em_ops(0), em_ops(1)):
                for op in op_pair:
                    op()

            # work queue: for pair j (2..15): dma for pair j+1... paced over
            # rounds [16(j-1), 16j)
            work = {}

            def add_work(r, fn):
                work.setdefault(max(1, r), []).append(fn)

            for j in range(1, NPAIR):
                base = NTP * (j - 1)
                if j + 1 < NPAIR:
                    qa, qb = (nc.sync, nc.gpsimd)
                    add_work(base, lambda t=2 * (j + 1), q=qa: dma_tile(t, q))
                    add_work(base + 1, lambda t=2 * (j + 1) + 1, q=qb: dma_tile(t, q))
                inter = [op for pair in zip(em_ops(2 * j), em_ops(2 * j + 1))
                         for op in pair]
                for i, op in enumerate(inter):
                    add_work(base + 2 + i * 10 // 16, op)

            # partial escan-out DMAs (top/bottom halves done by then)
            add_work(140, lambda: nc.scalar.dma_start(
                e_top[:, 0:HALF * NB // 2], escan2[0:TA, 0:HALF * NB // 2]))
            add_work(150, lambda: nc.scalar.dma_start(
                e_bot[:, 0:HALF * NB // 2], escan2[BOT:H113, 0:HALF * NB // 2]))

            # ---- the fused chain ----
            x = x1
            for i in range(1, HALF):
                for fn in work.get(i, ()):
                    fn()
                y = psY.tile([H113, NB], F32, tag="y", name="y")
                nc.tensor.matmul(y[:], mhat_sb[:], x[:], start=True, stop=True)
                xn = xpool.tile([H113, NB], BF16, tag="x", name="xn")
                nc.vector.tensor_tensor(xn[:], y[:],
                                        escan2[:, i * NB:(i + 1) * NB],
                                        mybir.AluOpType.mult)
                x = xn

            # final round: u_256 at partitions 64:113, dot with rb_256
            y = psY.tile([H113, NB], F32, tag="y", name="y")
            nc.tensor.matmul(y[:], mfin_sb[:], x[:], start=True, stop=True)
            g = consts.tile([H113, NB], BF16, tag="g")
            nc.vector.tensor_tensor(g[BOT:H113, :], y[BOT:H113, :],
                                    x[BOT:H113, :], mybir.AluOpType.mult)
            zp = psZ.tile([1, NB], F32, tag="zp")
            nc.tensor.matmul(zp[:], onesz_sb[BOT:H113, :], g[BOT:H113, :],
                             start=True, stop=True)
            zsb = consts.tile([1, NB], F32, tag="zsb")
            nc.vector.tensor_copy(zsb[:], zp[:])
            nc.sync.dma_start(z_out, zsb[:])
            nc.scalar.dma_start(e_top[:, HALF * NB // 2:],
                                escan2[0:TA, HALF * NB // 2:])
            nc.gpsimd.dma_start(e_bot[:, HALF * NB // 2:],
                                escan2[BOT:H113, HALF * NB // 2:])

    nc.compile()
    return nc


def _host_inputs(H, W, bb, st, en, tr, tag, s_len, w_mask):
    A = np.exp(tr.astype(np.float64)).astype(np.float32)
    Ahat = np.zeros((TA, TA), np.float32)
    Ahat[:T, :T] = A
    Ahat[:T, T] = np.exp(en).astype(np.float32)
    Ahat[T, T] = 1.0

    import ml_dtypes
    BF = ml_dtypes.bfloat16
    mhat = np.zeros((H113, H113), np.float32)
    mhat[:TA, :TA] = Ahat
    mhat[BOT:, BOT:] = Ahat.T
    mfin = np.zeros((H113, H113), np.float32)
    mfin[:TA, BOT:] = Ahat

    beta_init = np.zeros((TA, NB), np.float32)
    beta_init[:T, :] = np.exp(en).astype(np.float32)[:, None]
    beta_init[T, :] = 1.0
    NEGb = np.float32(np.float16(NEGH))

    Wp = np.zeros((U, TA), np.float16)
    Wp[:, :T] = W.astype(np.float16)
    onesk = np.ones((1, TA), np.float16)
    onesk[0, T] = -1.0

    perm = np.concatenate([np.arange(HALF), np.arange(S - 1, HALF - 1, -1)])

    shared = {
        "w": Wp,
        "mhat": mhat.astype(BF),
        "mfin": mfin.astype(BF),
        "onesk": onesk,
        "onesz": np.ones((TA, 1), BF),
        "bias_e": np.concatenate([(bb - C0).astype(np.float32),
                                  [NEGb]]).reshape(TA, 1),
        "bias_a0": np.concatenate([(bb + st).astype(np.float32),
                                   [np.float32(NEG)]]).reshape(TA, 1),
        "beta_init": beta_init.astype(BF),
    }

    s_idx = np.arange(S)
    in_maps = []
    for k in range(NCORES):
        rows = slice(k * NB, (k + 1) * NB)
        len_l = s_len[rows]
        pad = (s_idx[None, :] >= len_l[:, None])          # (NB, S)
        padflag = np.where(pad, NEGb, np.float32(0.0)).T[perm]  # (S, NB)
        im = dict(shared)
        im["h2"] = np.ascontiguousarray(
            H[rows][:, perm].transpose(2, 1, 0).astype(np.float16))
        im["padflag"] = np.ascontiguousarray(
            padflag.reshape(1, NPOS).astype(np.float16))
        in_maps.append(im)
    return in_maps


def kernel(H, W, b, start_transitions, end_transitions, transitions,
           tag, s_len, w_mask):
    global _PROGRAM
    H = np.asarray(H, np.float32)
    W = np.asarray(W, np.float32)
    bb = np.asarray(b, np.float32)
    st = np.asarray(start_transitions, np.float32)
    en = np.asarray(end_transitions, np.float32)
    tr = np.asarray(transitions, np.float32)
    tag = np.asarray(tag)
    s_len = np.asarray(s_len)
    w_mask = np.asarray(w_mask, np.float32)

    if _PROGRAM is None:
        _PROGRAM = _build_program()
    nc = _PROGRAM

    in_maps = _host_inputs(H, W, bb, st, en, tr, tag, s_len, w_mask)
    trace = bool(int(os.environ.get("KERNEL_TRACE", "0")))
    r = run_bass_kernel_spmd(nc, in_maps, list(range(NCORES)), trace=trace,
                             tmpdir=os.environ.get("KERNEL_TRACE_DIR") or None)
    global LAST_EXEC_NS, LAST_RESULT
    LAST_RESULT = r
    LAST_EXEC_NS = r.exec_time_ns
    res = r.results

    z = np.concatenate([np.asarray(rr["z_out"]).reshape(NB) for rr in res])
    etop = np.stack([np.asarray(rr["e_top"]) for rr in res])  # (NC,TA,HALF*NB)
    ebot = np.stack([np.asarray(rr["e_bot"]) for rr in res])

    # ---- host assembly ----
    perm = np.concatenate([np.arange(HALF), np.arange(S - 1, HALF - 1, -1)])
    logZ = np.log(z.astype(np.float64)) + C0 * (s_len.astype(np.float64) - 1)

    # scores[tag] = log(escan[tag]) - (b[tag] - C0) at unpadded positions
    e_all = np.zeros((NCORES, TA, S, NB), np.float32)
    e_all[:, :, :HALF] = etop.reshape(NCORES, TA, HALF, NB)
    e_all[:, :, perm[HALF:]] = ebot.reshape(NCORES, TA, HALF, NB)
    e_all = e_all.transpose(0, 3, 2, 1).reshape(B, S, TA)  # (B,S,TA)
    ge = np.take_along_axis(e_all, tag[..., None], axis=2)[..., 0]  # (B,S)
    ls = np.where(w_mask > 0, np.log(np.maximum(ge, 1e-30)), 0.0)
    num_emit = (ls.astype(np.float64)
                - (bb[tag].astype(np.float64) - C0) * w_mask).sum(axis=1)

    bidx = np.arange(B)
    num = (st[tag[:, 0]].astype(np.float64)
           + num_emit
           + (bb[tag].astype(np.float64) * w_mask).sum(axis=1)
           + (tr[tag[:, :-1], tag[:, 1:]].astype(np.float64) * w_mask[:, 1:]).sum(axis=1)
           + en[tag[bidx, s_len - 1]].astype(np.float64))
    return (num - logZ).astype(np.float32)


# revision 10
# speedup vs baseline: 1.1450x; 1.0049x over previous
"""Trainium2 Bass kernel for CRF log-likelihood (B=128, S=512, U=1024, T=48).

Strategy (data-parallel, 16 batch rows per core, no collectives):
  - Emission scores H@W on PE (K=1024 in 8 chunks of 128), f16 in / f32 psum,
    in 32 position-tiles of 16 positions (N=256 matmuls) so each emission
    matmul hides inside the chain matmul's pipeline-drain window.
  - Forward+backward scans fused: ONE block-diagonal (113x113) matmul per
    round (fwd states on partitions 0:49, bwd on 64:113 for 64-alignment)
    followed by ONE (113,16) DVE multiply with exp'd scores.  The bwd half
    of H is time-reversed on the host so both chains read the same escan
    column block each round.  256 rounds instead of 511.
  - A 49th "done" state absorbs finished rows (pad logits via a rank-1
    matmul), constant normalizer exp(-C0) per step, corrected on host.
  - Numerator: host recovers score[tag] = log(escan) - (b - C0) from the
    exp'd-scores tensor which is DMA'd out; no extra device work.
"""

import os
from collections import deque

import numpy as np

import concourse.bass as bass
import concourse.tile as tile
from concourse import bacc, mybir
from concourse.bass_utils import run_bass_kernel_spmd

B, S, U, T = 128, 512, 1024, 48
NCORES = 8
NB = B // NCORES          # 16 rows per core
NPOS = NB * S             # 8192 positions per core (pos = p*NB + b)
TA = T + 1                # 49 states (48 tags + "done")
BOT = 64                  # partition base of the bwd chain lane
H113 = BOT + TA           # 113
HALF = S // 2             # 256 positions per direction
NTP = 16                  # positions per emission tile
CPT = NTP * NB            # 256 columns per emission tile
NPAIR = HALF // NTP       # 16 tile pairs
C0 = 4.8                  # per-step log-space normalizer
NEG = -1.0e9
NEGH = -60000.0           # fp16-representable pad logit; exp() == 0
F32 = mybir.dt.float32
BF16 = mybir.dt.bfloat16
F16 = mybir.dt.float16

_PROGRAM = None
LAST_EXEC_NS = None
LAST_RESULT = None


def _build_program():
    nc = bacc.Bacc("TRN2", target_bir_lowering=False, debug=False,
                   enable_asserts=False)

    def din(name, shape, dt=F32):
        return nc.dram_tensor(name, list(shape), dt, kind="ExternalInput").ap()

    def dout(name, shape):
        return nc.dram_tensor(name, list(shape), F32, kind="ExternalOutput").ap()

    h2 = din("h2", (U, S, NB), F16)          # host-permuted (bwd half flipped)
    w = din("w", (U, TA), F16)               # 49th col zero
    mhat = din("mhat", (H113, H113), BF16)   # blockdiag lhsT
    mfin = din("mfin", (H113, H113), BF16)   # final-round lhsT
    onesk = din("onesk", (1, TA), F16)       # [1]*48 + [-1]
    onesz = din("onesz", (TA, 1), BF16)
    padflag = din("padflag", (1, NPOS), F16)  # {0, NEGH}, permuted
    bias_e = din("bias_e", (TA, 1))          # [b - C0; NEGb]
    bias_a0 = din("bias_a0", (TA, 1))        # [b + start; NEG]
    beta_init = din("beta_init", (TA, NB), BF16)

    z_out = dout("z_out", (1, NB))
    e_top = dout("e_top", (TA, HALF * NB))
    e_bot = dout("e_bot", (TA, HALF * NB))

    with tile.TileContext(nc) as tc:
        with (
            tc.tile_pool(name="consts", bufs=1) as consts,
            tc.tile_pool(name="hpool", bufs=16) as hpool,
            tc.tile_pool(name="epsum", bufs=4, space="PSUM") as epsum,
            tc.tile_pool(name="psY", bufs=2, space="PSUM") as psY,
            tc.tile_pool(name="psZ", bufs=1, space="PSUM") as psZ,
            tc.tile_pool(name="xpool", bufs=2) as xpool,
        ):
            # ---- critical-path constants first ----
            w_sb = consts.tile([128, 8 * TA], F16, tag="w_sb")
            nc.scalar.dma_start(w_sb[:].rearrange("p (c t) -> p c t", c=8),
                                w.rearrange("(c p) t -> p c t", p=128))
            padf_sb = consts.tile([1, NPOS], F16, tag="padf")
            nc.gpsimd.dma_start(padf_sb[:], padflag)

            hs_tiles = {}

            def dma_tile(t, q, halves=False):
                """t in 0..31: fwd tile j = 2j, bwd tile j = 2j+1."""
                j = t // 2
                p0 = (HALF if t % 2 else 0) + j * NTP
                hs = hpool.tile([128, 8 * CPT], F16, tag="hs", name="hs")
                hs_tiles[t] = hs
                src = h2[:, p0:p0 + NTP, :].rearrange(
                    "(c p) s b -> p c (s b)", p=128)
                dst = hs[:].rearrange("p (c x) -> p c x", c=8)
                if halves:
                    q[0].dma_start(dst[:, 0:4], src[:, 0:4])
                    q[1].dma_start(dst[:, 4:8], src[:, 4:8])
                else:
                    q.dma_start(dst, src)

            # first tile pair split in halves across four queues
            dma_tile(0, (nc.sync, nc.scalar), halves=True)
            dma_tile(1, (nc.gpsimd, nc.sync), halves=True)

            # ---- remaining constants ----
            mhat_sb = consts.tile([H113, H113], BF16, tag="mhat")
            nc.scalar.dma_start(mhat_sb[:], mhat)
            mfin_sb = consts.tile([H113, H113], BF16, tag="mfin")
            nc.scalar.dma_start(mfin_sb[:], mfin)
            onesk_sb = consts.tile([1, TA], F16, tag="onesk")
            nc.scalar.dma_start(onesk_sb[:], onesk)
            onesz_sb = consts.tile([H113, 1], BF16, tag="onesz")
            nc.scalar.dma_start(onesz_sb[BOT:H113, :], onesz)
            bias_e0_sb = consts.tile([TA, 1], F32, tag="bias_e0")
            nc.scalar.dma_start(bias_e0_sb[:], bias_e)
            bias_eb_sb = consts.tile([H113, 1], F32, tag="bias_eb")
            nc.scalar.dma_start(bias_eb_sb[BOT:H113, :], bias_e)
            bias_a0_sb = consts.tile([TA, 1], F32, tag="bias_a0")
            nc.scalar.dma_start(bias_a0_sb[:], bias_a0)
            beta_sb = consts.tile([H113, NB], BF16, tag="beta0")
            nc.scalar.dma_start(beta_sb[BOT:H113, :], beta_init)

            escan2 = consts.tile([H113, HALF * NB], F32, tag="escan2")
            # rows 49:64 must be zero; engine partition bases must be
            # 32-aligned, so clear 32:64 and let the fwd ACTs overwrite 32:49.
            # DVE is idle during the pre-phase.
            nc.vector.memset(escan2[32:BOT, :], 0.0)
            x1 = consts.tile([H113, NB], BF16, tag="x1")
            nc.gpsimd.memset(x1[:], 0.0)

            # remaining h tiles: big lookahead, two queues
            for t in range(2, 32):
                dma_tile(t, nc.sync if t % 2 == 0 else nc.gpsimd)

            def em_ops(t):
                """Emission ops for tile t as a list of closures."""
                j = t // 2
                is_b = t % 2
                pos0 = ((HALF if is_b else 0) + j * NTP) * NB
                cols = slice(j * NTP * NB, j * NTP * NB + CPT)
                lo, hi = (BOT, H113) if is_b else (0, TA)
                state = {}
                ops = []

                def mk_mm(hh):
                    def f():
                        if hh == 0:
                            state[0] = epsum.tile([H113, CPT], F32, tag="eps",
                                                  name="eps")
                        ps = state[0]
                        nc.tensor.matmul(ps[lo:hi, :],
                                         w_sb[:, hh * TA:(hh + 1) * TA],
                                         hs_tiles[t][:, hh * CPT:(hh + 1) * CPT],
                                         start=(hh == 0), stop=False)
                    return f

                def mk_pad():
                    def f():
                        nc.tensor.matmul(state[0][lo:hi, :], onesk_sb[:],
                                         padf_sb[:, pos0:pos0 + CPT],
                                         start=False, stop=True)
                    return f

                def mk_act():
                    def f():
                        ps = state[0]
                        if is_b:
                            nc.scalar.activation(
                                escan2[BOT:H113, cols], ps[BOT:H113, :],
                                mybir.ActivationFunctionType.Exp,
                                bias=bias_eb_sb[BOT:H113, :])
                            if t == 1:
                                nc.vector.tensor_tensor(
                                    x1[BOT:H113, :], beta_sb[BOT:H113, :],
                                    escan2[BOT:H113, 0:NB],
                                    mybir.AluOpType.mult)
                        else:
                            nc.scalar.activation(
                                escan2[0:TA, cols], ps[0:TA, :],
                                mybir.ActivationFunctionType.Exp,
                                bias=bias_e0_sb[:])
                            if t == 0:
                                nc.scalar.activation(
                                    x1[0:TA, :], ps[0:TA, 0:NB],
                                    mybir.ActivationFunctionType.Exp,
                                    bias=bias_a0_sb[:])
                    return f

                for hh in range(8):
                    ops.append(mk_mm(hh))
                ops.append(mk_pad())
                ops.append(mk_act())
                return ops

            # ---- emission work, floored onto the chain timeline ----
            # pair 0 unfloored (pre-phase); pair j floored at T0 + R*16*(j-1)
            T0_US = 10.0
            R_US = 0.40
            for op_pair in zip(em_ops(0), em_ops(1)):
                for op in op_pair:
                    op()
            for j in range(1, NPAIR):
                floor_ms = (T0_US + R_US * NTP * (j - 1)) * 1e-3
                with tc.tile_wait_until(floor_ms):
                    for op_pair in zip(em_ops(2 * j), em_ops(2 * j + 1)):
                        for op in op_pair:
                            op()

            # partial escan-out DMAs (first halves ready mid-chain)
            with tc.tile_wait_until(0.070):
                nc.scalar.dma_start(e_top[:, 0:HALF * NB // 2],
                                    escan2[0:TA, 0:HALF * NB // 2])
                nc.scalar.dma_start(e_bot[:, 0:HALF * NB // 2],
                                    escan2[BOT:H113, 0:HALF * NB // 2])

            # ---- the fused chain ----
            x = x1
            for i in range(1, HALF):
                y = psY.tile([H113, NB], F32, tag="y", name="y")
                nc.tensor.matmul(y[:], mhat_sb[:], x[:], start=True, stop=True)
                xn = xpool.tile([H113, NB], BF16, tag="x", name="xn")
                nc.vector.tensor_tensor(xn[:], y[:],
                                        escan2[:, i * NB:(i + 1) * NB],
                                        mybir.AluOpType.mult)
                x = xn

            # final round: u_256 at partitions 64:113, dot with rb_256
            y = psY.tile([H113, NB], F32, tag="y", name="y")
            nc.tensor.matmul(y[:], mfin_sb[:], x[:], start=True, stop=True)
            g = consts.tile([H113, NB], BF16, tag="g")
            nc.vector.tensor_tensor(g[BOT:H113, :], y[BOT:H113, :],
                                    x[BOT:H113, :], mybir.AluOpType.mult)
            zp = psZ.tile([1, NB], F32, tag="zp")
            nc.tensor.matmul(zp[:], onesz_sb[BOT:H113, :], g[BOT:H113, :],
                             start=True, stop=True)
            zsb = consts.tile([1, NB], F32, tag="zsb")
            nc.vector.tensor_copy(zsb[:], zp[:])
            nc.sync.dma_start(z_out, zsb[:])
            nc.scalar.dma_start(e_top[:, HALF * NB // 2:],
                                escan2[0:TA, HALF * NB // 2:])
            nc.gpsimd.dma_start(e_bot[:, HALF * NB // 2:],
                                escan2[BOT:H113, HALF * NB // 2:])

    nc.compile()
    return nc


def _host_inputs(H, W, bb, st, en, tr, tag, s_len, w_mask):
    A = np.exp(tr.astype(np.float64)).astype(np.float32)
    Ahat = np.zeros((TA, TA), np.float32)
    Ahat[:T, :T] = A
    Ahat[:T, T] = np.exp(en).astype(np.float32)
    Ahat[T, T] = 1.0

    import ml_dtypes
    BF = ml_dtypes.bfloat16
    mhat = np.zeros((H113, H113), np.float32)
    mhat[:TA, :TA] = Ahat
    mhat[BOT:, BOT:] = Ahat.T
    mfin = np.zeros((H113, H113), np.float32)
    mfin[:TA, BOT:] = Ahat

    beta_init = np.zeros((TA, NB), np.float32)
    beta_init[:T, :] = np.exp(en).astype(np.float32)[:, None]
    beta_init[T, :] = 1.0
    NEGb = np.float32(np.float16(NEGH))

    Wp = np.zeros((U, TA), np.float16)
    Wp[:, :T] = W.astype(np.float16)
    onesk = np.ones((1, TA), np.float16)
    onesk[0, T] = -1.0

    perm = np.concatenate([np.arange(HALF), np.arange(S - 1, HALF - 1, -1)])

    shared = {
        "w": Wp,
        "mhat": mhat.astype(BF),
        "mfin": mfin.astype(BF),
        "onesk": onesk,
        "onesz": np.ones((TA, 1), BF),
        "bias_e": np.concatenate([(bb - C0).astype(np.float32),
                                  [NEGb]]).reshape(TA, 1),
        "bias_a0": np.concatenate([(bb + st).astype(np.float32),
                                   [np.float32(NEG)]]).reshape(TA, 1),
        "beta_init": beta_init.astype(BF),
    }

    s_idx = np.arange(S)
    in_maps = []
    for k in range(NCORES):
        rows = slice(k * NB, (k + 1) * NB)
        len_l = s_len[rows]
        pad = (s_idx[None, :] >= len_l[:, None])          # (NB, S)
        padflag = np.where(pad, NEGb, np.float32(0.0)).T[perm]  # (S, NB)
        im = dict(shared)
        im["h2"] = np.ascontiguousarray(
            H[rows][:, perm].transpose(2, 1, 0).astype(np.float16))
        im["padflag"] = np.ascontiguousarray(
            padflag.reshape(1, NPOS).astype(np.float16))
        in_maps.append(im)
    return in_maps


def kernel(H, W, b, start_transitions, end_transitions, transitions,
           tag, s_len, w_mask):
    global _PROGRAM
    H = np.asarray(H, np.float32)
    W = np.asarray(W, np.float32)
    bb = np.asarray(b, np.float32)
    st = np.asarray(start_transitions, np.float32)
    en = np.asarray(end_transitions, np.float32)
    tr = np.asarray(transitions, np.float32)
    tag = np.asarray(tag)
    s_len = np.asarray(s_len)
    w_mask = np.asarray(w_mask, np.float32)

    if _PROGRAM is None:
        _PROGRAM = _build_program()
    nc = _PROGRAM

    in_maps = _host_inputs(H, W, bb, st, en, tr, tag, s_len, w_mask)
    trace = bool(int(os.environ.get("KERNEL_TRACE", "0")))
    r = run_bass_kernel_spmd(nc, in_maps, list(range(NCORES)), trace=trace,
                             tmpdir=os.environ.get("KERNEL_TRACE_DIR") or None)
    global LAST_EXEC_NS, LAST_RESULT
    LAST_RESULT = r
    LAST_EXEC_NS = r.exec_time_ns
    res = r.results

    z = np.concatenate([np.asarray(rr["z_out"]).reshape(NB) for rr in res])
    etop = np.stack([np.asarray(rr["e_top"]) for rr in res])  # (NC,TA,HALF*NB)
    ebot = np.stack([np.asarray(rr["e_bot"]) for rr in res])

    # ---- host assembly ----
    perm = np.concatenate([np.arange(HALF), np.arange(S - 1, HALF - 1, -1)])
    logZ = np.log(z.astype(np.float64)) + C0 * (s_len.astype(np.float64) - 1)

    # scores[tag] = log(escan[tag]) - (b[tag] - C0) at unpadded positions
    e_all = np.zeros((NCORES, TA, S, NB), np.float32)
    e_all[:, :, :HALF] = etop.reshape(NCORES, TA, HALF, NB)
    e_all[:, :, perm[HALF:]] = ebot.reshape(NCORES, TA, HALF, NB)
    e_all = e_all.transpose(0, 3, 2, 1).reshape(B, S, TA)  # (B,S,TA)
    ge = np.take_along_axis(e_all, tag[..., None], axis=2)[..., 0]  # (B,S)
    ls = np.where(w_mask > 0, np.log(np.maximum(ge, 1e-30)), 0.0)
    num_emit = (ls.astype(np.float64)
                - (bb[tag].astype(np.float64) - C0) * w_mask).sum(axis=1)

    bidx = np.arange(B)
    num = (st[tag[:, 0]].astype(np.float64)
           + num_emit
           + (bb[tag].astype(np.float64) * w_mask).sum(axis=1)
           + (tr[tag[:, :-1], tag[:, 1:]].astype(np.float64) * w_mask[:, 1:]).sum(axis=1)
           + en[tag[bidx, s_len - 1]].astype(np.float64))
    return (num - logZ).astype(np.float32)


# revision 11
# speedup vs baseline: 1.2137x; 1.0599x over previous
"""Trainium2 Bass kernel for CRF log-likelihood (B=128, S=512, U=1024, T=48).

Strategy (data-parallel, 16 batch rows per core, no collectives):
  - Emission scores H@W on PE (K=1024 in 8 chunks of 128), f16 in / f32 psum,
    in 32 position-tiles of 16 positions (N=256 matmuls) so each emission
    matmul hides inside the chain matmul's pipeline-drain window.
  - Forward+backward scans fused: ONE block-diagonal (113x113) matmul per
    round (fwd states on partitions 0:49, bwd on 64:113 for 64-alignment)
    followed by ONE (113,16) DVE multiply with exp'd scores.  The bwd half
    of H is time-reversed on the host so both chains read the same escan
    column block each round.  256 rounds instead of 511.
  - A 49th "done" state absorbs finished rows (pad logits via a rank-1
    matmul), constant normalizer exp(-C0) per step, corrected on host.
  - Numerator: host recovers score[tag] = log(escan) - (b - C0) from the
    exp'd-scores tensor which is DMA'd out; no extra device work.
"""

import os
from collections import deque

import numpy as np

import concourse.bass as bass
import concourse.tile as tile
from concourse import bacc, mybir
from concourse.bass_utils import run_bass_kernel_spmd

B, S, U, T = 128, 512, 1024, 48
NCORES = 8
NB = B // NCORES          # 16 rows per core
NPOS = NB * S             # 8192 positions per core (pos = p*NB + b)
TA = T + 1                # 49 states (48 tags + "done")
BOT = 64                  # partition base of the bwd chain lane
H113 = BOT + TA           # 113
HALF = S // 2             # 256 positions per direction
NTP = 16                  # positions per emission tile
CPT = NTP * NB            # 256 columns per emission tile
NPAIR = HALF // NTP       # 16 tile pairs
C0 = 4.8                  # per-step log-space normalizer
NEG = -1.0e9
NEGH = -60000.0           # fp16-representable pad logit; exp() == 0
F32 = mybir.dt.float32
BF16 = mybir.dt.bfloat16
F16 = mybir.dt.float16

_PROGRAM = None
LAST_EXEC_NS = None
LAST_RESULT = None


def _build_program():
    nc = bacc.Bacc("TRN2", target_bir_lowering=False, debug=False,
                   enable_asserts=False)

    def din(name, shape, dt=F32):
        return nc.dram_tensor(name, list(shape), dt, kind="ExternalInput").ap()

    def dout(name, shape):
        return nc.dram_tensor(name, list(shape), F32, kind="ExternalOutput").ap()

    h2 = din("h2", (U, S, NB), F16)          # host-permuted (bwd half flipped)
    w = din("w", (U, TA), F16)               # 49th col zero
    mhat = din("mhat", (H113, H113), BF16)   # blockdiag lhsT
    mfin = din("mfin", (H113, H113), BF16)   # final-round lhsT
    onesk = din("onesk", (1, TA), F16)       # [1]*48 + [-1]
    onesz = din("onesz", (TA, 1), BF16)
    padflag = din("padflag", (1, NPOS), F16)  # {0, NEGH}, permuted
    bias_e = din("bias_e", (TA, 1))          # [b - C0; NEGb]
    bias_a0 = din("bias_a0", (TA, 1))        # [b + start; NEG]
    beta_init = din("beta_init", (TA, NB), BF16)

    z_out = dout("z_out", (1, NB))
    e_top = dout("e_top", (TA, HALF * NB))
    e_bot = dout("e_bot", (TA, HALF * NB))

    with tile.TileContext(nc) as tc:
        with (
            tc.tile_pool(name="consts", bufs=1) as consts,
            tc.tile_pool(name="hpool", bufs=16) as hpool,
            tc.tile_pool(name="epsum", bufs=4, space="PSUM") as epsum,
            tc.tile_pool(name="psY", bufs=2, space="PSUM") as psY,
            tc.tile_pool(name="psZ", bufs=1, space="PSUM") as psZ,
            tc.tile_pool(name="xpool", bufs=2) as xpool,
        ):
            # ---- critical-path constants first ----
            w_sb = consts.tile([128, 8 * TA], F16, tag="w_sb")
            nc.scalar.dma_start(w_sb[:].rearrange("p (c t) -> p c t", c=8),
                                w.rearrange("(c p) t -> p c t", p=128))
            padf_sb = consts.tile([1, NPOS], F16, tag="padf")
            nc.gpsimd.dma_start(padf_sb[:], padflag)

            hs_tiles = {}

            def dma_tile(t, q, halves=False):
                """t in 0..31: fwd tile j = 2j, bwd tile j = 2j+1."""
                j = t // 2
                p0 = (HALF if t % 2 else 0) + j * NTP
                hs = hpool.tile([128, 8 * CPT], F16, tag="hs", name="hs")
                hs_tiles[t] = hs
                src = h2[:, p0:p0 + NTP, :].rearrange(
                    "(c p) s b -> p c (s b)", p=128)
                dst = hs[:].rearrange("p (c x) -> p c x", c=8)
                if halves:
                    q[0].dma_start(dst[:, 0:4], src[:, 0:4])
                    q[1].dma_start(dst[:, 4:8], src[:, 4:8])
                else:
                    q.dma_start(dst, src)

            # first tile pair split in halves across four queues
            dma_tile(0, (nc.sync, nc.scalar), halves=True)
            dma_tile(1, (nc.gpsimd, nc.sync), halves=True)

            # ---- remaining constants ----
            mhat_sb = consts.tile([H113, H113], BF16, tag="mhat")
            nc.scalar.dma_start(mhat_sb[:], mhat)
            mfin_sb = consts.tile([H113, H113], BF16, tag="mfin")
            nc.scalar.dma_start(mfin_sb[:], mfin)
            onesk_sb = consts.tile([1, TA], F16, tag="onesk")
            nc.scalar.dma_start(onesk_sb[:], onesk)
            onesz_sb = consts.tile([H113, 1], BF16, tag="onesz")
            nc.scalar.dma_start(onesz_sb[BOT:H113, :], onesz)
            bias_e0_sb = consts.tile([TA, 1], F32, tag="bias_e0")
            nc.scalar.dma_start(bias_e0_sb[:], bias_e)
            bias_eb_sb = consts.tile([H113, 1], F32, tag="bias_eb")
            nc.scalar.dma_start(bias_eb_sb[BOT:H113, :], bias_e)
            bias_a0_sb = consts.tile([TA, 1], F32, tag="bias_a0")
            nc.scalar.dma_start(bias_a0_sb[:], bias_a0)
            beta_sb = consts.tile([H113, NB], BF16, tag="beta0")
            nc.scalar.dma_start(beta_sb[BOT:H113, :], beta_init)

            escan2 = consts.tile([H113, HALF * NB], F32, tag="escan2")
            # rows 49:64 must be zero; engine partition bases must be
            # 32-aligned, so clear 32:64 and let the fwd ACTs overwrite 32:49.
            # DVE is idle during the pre-phase.
            nc.vector.memset(escan2[32:BOT, :], 0.0)
            x1 = consts.tile([H113, NB], BF16, tag="x1")
            nc.gpsimd.memset(x1[:], 0.0)

            # remaining h tiles: big lookahead, two queues
            for t in range(2, 32):
                dma_tile(t, nc.sync if t % 2 == 0 else nc.gpsimd)

            def em_ops(t, sp0=0, np_=NTP):
                """Emission ops for positions [sp0, sp0+np_) of tile t."""
                j = t // 2
                is_b = t % 2
                pos0 = ((HALF if is_b else 0) + j * NTP + sp0) * NB
                c0 = (j * NTP + sp0) * NB
                ncol = np_ * NB
                cols = slice(c0, c0 + ncol)
                lo, hi = (BOT, H113) if is_b else (0, TA)
                state = {}
                ops = []

                def mk_mm(hh):
                    def f():
                        if hh == 0:
                            state[0] = epsum.tile([H113, CPT], F32, tag="eps",
                                                  name="eps")
                        ps = state[0]
                        off = hh * CPT + sp0 * NB
                        nc.tensor.matmul(ps[lo:hi, 0:ncol],
                                         w_sb[:, hh * TA:(hh + 1) * TA],
                                         hs_tiles[t][:, off:off + ncol],
                                         start=(hh == 0), stop=False)
                    return f

                def mk_pad():
                    def f():
                        nc.tensor.matmul(state[0][lo:hi, 0:ncol], onesk_sb[:],
                                         padf_sb[:, pos0:pos0 + ncol],
                                         start=False, stop=True)
                    return f

                def mk_act():
                    def f():
                        ps = state[0]
                        if is_b:
                            nc.scalar.activation(
                                escan2[BOT:H113, cols], ps[BOT:H113, 0:ncol],
                                mybir.ActivationFunctionType.Exp,
                                bias=bias_eb_sb[BOT:H113, :])
                            if t == 1 and sp0 == 0:
                                nc.vector.tensor_tensor(
                                    x1[BOT:H113, :], beta_sb[BOT:H113, :],
                                    escan2[BOT:H113, 0:NB],
                                    mybir.AluOpType.mult)
                        else:
                            nc.scalar.activation(
                                escan2[0:TA, cols], ps[0:TA, 0:ncol],
                                mybir.ActivationFunctionType.Exp,
                                bias=bias_e0_sb[:])
                            if t == 0 and sp0 == 0:
                                nc.scalar.activation(
                                    x1[0:TA, :], ps[0:TA, 0:NB],
                                    mybir.ActivationFunctionType.Exp,
                                    bias=bias_a0_sb[:])
                    return f

                for hh in range(8):
                    ops.append(mk_mm(hh))
                ops.append(mk_pad())
                ops.append(mk_act())
                return ops

            # ---- pre-chain: pair 0 in 8-position subtiles (fast start) ----
            for op_pair in zip(em_ops(0, 0, 8), em_ops(1, 0, 8)):
                for op in op_pair:
                    op()
            for op_pair in zip(em_ops(0, 8, 8), em_ops(1, 8, 8)):
                for op in op_pair:
                    op()

            # emission work for pairs 1..15 paced into chain rounds with
            # sim-time floors (just under the scheduler's chain pace)
            T0_US = 7.0
            R_US = 0.30
            work = {}

            def add_work(r, fn):
                work.setdefault(max(1, min(HALF - 1, r)), []).append(fn)

            for j in range(1, NPAIR):
                base = NTP * (j - 1)
                inter = [op for pair in zip(em_ops(2 * j), em_ops(2 * j + 1))
                         for op in pair]
                for i, op in enumerate(inter):
                    add_work(base + 2 + i * 12 // 20, op)

            # partial escan-out DMAs on the gpsimd queue mid-chain
            add_work(150, lambda: nc.gpsimd.dma_start(
                e_top[:, 0:HALF * NB // 2], escan2[0:TA, 0:HALF * NB // 2]))
            add_work(154, lambda: nc.gpsimd.dma_start(
                e_bot[:, 0:HALF * NB // 2], escan2[BOT:H113, 0:HALF * NB // 2]))

            # ---- the fused chain ----
            x = x1
            for i in range(1, HALF):
                if i in work:
                    floor_ms = (T0_US + R_US * i) * 1e-3
                    with tc.tile_wait_until(floor_ms):
                        for fn in work[i]:
                            fn()
                y = psY.tile([H113, NB], F32, tag="y", name="y")
                nc.tensor.matmul(y[:], mhat_sb[:], x[:], start=True, stop=True)
                xn = xpool.tile([H113, NB], BF16, tag="x", name="xn")
                nc.vector.tensor_tensor(xn[:], y[:],
                                        escan2[:, i * NB:(i + 1) * NB],
                                        mybir.AluOpType.mult)
                x = xn

            # final round: u_256 at partitions 64:113, dot with rb_256
            y = psY.tile([H113, NB], F32, tag="y", name="y")
            nc.tensor.matmul(y[:], mfin_sb[:], x[:], start=True, stop=True)
            g = consts.tile([H113, NB], BF16, tag="g")
            nc.vector.tensor_tensor(g[BOT:H113, :], y[BOT:H113, :],
                                    x[BOT:H113, :], mybir.AluOpType.mult)
            zp = psZ.tile([1, NB], F32, tag="zp")
            nc.tensor.matmul(zp[:], onesz_sb[BOT:H113, :], g[BOT:H113, :],
                             start=True, stop=True)
            zsb = consts.tile([1, NB], F32, tag="zsb")
            nc.vector.tensor_copy(zsb[:], zp[:])
            nc.sync.dma_start(z_out, zsb[:])
            nc.scalar.dma_start(e_top[:, HALF * NB // 2:],
                                escan2[0:TA, HALF * NB // 2:])
            nc.gpsimd.dma_start(e_bot[:, HALF * NB // 2:],
                                escan2[BOT:H113, HALF * NB // 2:])

    nc.compile()
    return nc


def _host_inputs(H, W, bb, st, en, tr, tag, s_len, w_mask):
    A = np.exp(tr.astype(np.float64)).astype(np.float32)
    Ahat = np.zeros((TA, TA), np.float32)
    Ahat[:T, :T] = A
    Ahat[:T, T] = np.exp(en).astype(np.float32)
    Ahat[T, T] = 1.0

    import ml_dtypes
    BF = ml_dtypes.bfloat16
    mhat = np.zeros((H113, H113), np.float32)
    mhat[:TA, :TA] = Ahat
    mhat[BOT:, BOT:] = Ahat.T
    mfin = np.zeros((H113, H113), np.float32)
    mfin[:TA, BOT:] = Ahat

    beta_init = np.zeros((TA, NB), np.float32)
    beta_init[:T, :] = np.exp(en).astype(np.float32)[:, None]
    beta_init[T, :] = 1.0
    NEGb = np.float32(np.float16(NEGH))

    Wp = np.zeros((U, TA), np.float16)
    Wp[:, :T] = W.astype(np.float16)
    onesk = np.ones((1, TA), np.float16)
    onesk[0, T] = -1.0

    perm = np.concatenate([np.arange(HALF), np.arange(S - 1, HALF - 1, -1)])

    shared = {
        "w": Wp,
        "mhat": mhat.astype(BF),
        "mfin": mfin.astype(BF),
        "onesk": onesk,
        "onesz": np.ones((TA, 1), BF),
        "bias_e": np.concatenate([(bb - C0).astype(np.float32),
                                  [NEGb]]).reshape(TA, 1),
        "bias_a0": np.concatenate([(bb + st).astype(np.float32),
                                   [np.float32(NEG)]]).reshape(TA, 1),
        "beta_init": beta_init.astype(BF),
    }

    s_idx = np.arange(S)
    in_maps = []
    for k in range(NCORES):
        rows = slice(k * NB, (k + 1) * NB)
        len_l = s_len[rows]
        pad = (s_idx[None, :] >= len_l[:, None])          # (NB, S)
        padflag = np.where(pad, NEGb, np.float32(0.0)).T[perm]  # (S, NB)
        im = dict(shared)
        im["h2"] = np.ascontiguousarray(
            H[rows][:, perm].transpose(2, 1, 0).astype(np.float16))
        im["padflag"] = np.ascontiguousarray(
            padflag.reshape(1, NPOS).astype(np.float16))
        in_maps.append(im)
    return in_maps


def kernel(H, W, b, start_transitions, end_transitions, transitions,
           tag, s_len, w_mask):
    global _PROGRAM
    H = np.asarray(H, np.float32)
    W = np.asarray(W, np.float32)
    bb = np.asarray(b, np.float32)
    st = np.asarray(start_transitions, np.float32)
    en = np.asarray(end_transitions, np.float32)
    tr = np.asarray(transitions, np.float32)
    tag = np.asarray(tag)
    s_len = np.asarray(s_len)
    w_mask = np.asarray(w_mask, np.float32)

    if _PROGRAM is None:
        _PROGRAM = _build_program()
    nc = _PROGRAM

    in_maps = _host_inputs(H, W, bb, st, en, tr, tag, s_len, w_mask)
    trace = bool(int(os.environ.get("KERNEL_TRACE", "0")))
    r = run_bass_kernel_spmd(nc, in_maps, list(range(NCORES)), trace=trace,
                             tmpdir=os.environ.get("KERNEL_TRACE_DIR") or None)
    global LAST_EXEC_NS, LAST_RESULT
    LAST_RESULT = r
    LAST_EXEC_NS = r.exec_time_ns
    res = r.results

    z = np.concatenate([np.asarray(rr["z_out"]).reshape(NB) for rr in res])
    etop = np.stack([np.asarray(rr["e_top"]) for rr in res])  # (NC,TA,HALF*NB)
    ebot = np.stack([np.asarray(rr["e_bot"]) for rr in res])

    # ---- host assembly ----
    perm = np.concatenate([np.arange(HALF), np.arange(S - 1, HALF - 1, -1)])
    logZ = np.log(z.astype(np.float64)) + C0 * (s_len.astype(np.float64) - 1)

    # scores[tag] = log(escan[tag]) - (b[tag] - C0) at unpadded positions
    e_all = np.zeros((NCORES, TA, S, NB), np.float32)
    e_all[:, :, :HALF] = etop.reshape(NCORES, TA, HALF, NB)
    e_all[:, :, perm[HALF:]] = ebot.reshape(NCORES, TA, HALF, NB)
    e_all = e_all.transpose(0, 3, 2, 1).reshape(B, S, TA)  # (B,S,TA)
    ge = np.take_along_axis(e_all, tag[..., None], axis=2)[..., 0]  # (B,S)
    ls = np.where(w_mask > 0, np.log(np.maximum(ge, 1e-30)), 0.0)
    num_emit = (ls.astype(np.float64)
                - (bb[tag].astype(np.float64) - C0) * w_mask).sum(axis=1)

    bidx = np.arange(B)
    num = (st[tag[:, 0]].astype(np.float64)
           + num_emit
           + (bb[tag].astype(np.float64) * w_mask).sum(axis=1)
           + (tr[tag[:, :-1], tag[:, 1:]].astype(np.float64) * w_mask[:, 1:]).sum(axis=1)
           + en[tag[bidx, s_len - 1]].astype(np.float64))
    return (num - logZ).astype(np.float32)
